# revision 34
# baseline (speedup 1.0000x reference)
"""Self-contained Trainium2 attention-block kernel (8 NeuronCores, SPMD).

Problem: x[4,4096,128], Wq/Wk[64,128], Wv[128,128] ->
  softmax((x Wq^T)(x Wk^T)^T / 8) (x Wv^T)   -> [4,4096,128] f32

Sharding: data-parallel over batch (4) x query-halves (2) = 8 cores.
Each core: q rows 2048, full K (4096) recomputed locally. No collectives.

v2 design (v1 measured 97.3us here / 99.2us on the grading harness;
this version measured 97.4us here with a colder-device-friendly start):
  - P*V reassociated as (P*X)*Wv^T: the per-chunk PV matmul uses raw x
    chunks as stationary weights, killing the V projection.
  - k-rotation: host rotates the k rows by h*2048 per core so the core's
    own q rows are always xT[:, 0:2048] -> no separate xqT tensor, one
    SPMD program for all cores (k-order is softmax/PX-invariant).
  - prologue: 9 consolidated sync-ring DMAs in consumption order (a
    second HWDGE ring was tried and let bulk transfers steal HBM
    bandwidth from the critical first slices -> slower).
  - PE warm-up: 8 dummy accumulating matmuls during the input-DMA wait
    trip the HAM activity window so the real stream starts at 2.4 GHz.
  - exp split across engines: 4 of 16 groups per q-block run on the DVE
    as a one-instruction Schraudolph bit-trick (tensor_scalar u16 =
    s*A + B, bitcast to bf16 ~= exp(s/8), ~1.2% weight rms err); the
    other 12 use ScalarE's table exp.
  - softmax denominator: DVE chunk-pair adds (t1), GpSimd pair sums
    (t2, emitted at lag 1), accumulating ones-matmuls on PE folding the
    128 k-partitions into a broadcast D row, emitted at lag 3 at the
    END of the slot body -- the cross-engine t1->t2->D chain otherwise
    head-of-line blocks the strict-FIFO PE queue (~10us of stalls).
  - finish: recip (DVE) -> px cast to bf16 (ScalarE) -> pxn mul (DVE)
    -> Wv matmul -> cast -> DMA; the last block is split into column
    halves so the tail chain pipelines across engines.
Measured: v2.3 (4 DVE slabs) 97408/97440/98346ns; v2.8 (+4
ramp-fill matmuls between the prologue projections and the first ST
groups) 96490ns; v3.0 (this config: 5 pair-second DVE slabs
(1,5,9,11,13), balancing Sc 59.1us vs DVE 60.9us busy) 93952ns, fro
rel err 4.52e-3.  HAM reaches K=8/8 at ~10.2us, right as the real
stream starts.  Near-misses: 6 ramp-fill matmuls 97304; T2_LAG=0 +
DMM_LAG=4 + FINISH_DELAY=5 regress to 110355 (the D-chain lags are a
razor-edge optimum at 1/3/4).
Schedule sensitivities found: whole-slab exp assignment beats per-slab
column-splitting (the split's DVE-queue serialization delays the st
psum WAR release and the t1 chain); DVE exp groups must be pair-second
(odd) positions or the t2 chain couples in; JIT projections must NOT
rotate through the st psum ring (off-by-one ring WAR stalls ~7us).
"""

import sys

sys.path.insert(0, "/opt/trn_rl_repo")

from contextlib import ExitStack

import ml_dtypes
import numpy as np

import concourse.bass as bass  # noqa: F401
import concourse.bacc as bacc
import concourse.tile as tile
from concourse import mybir
from concourse.bass_utils import run_bass_kernel_spmd

BF16 = mybir.dt.bfloat16
F32 = mybir.dt.float32
U16 = mybir.dt.uint16
NPBF16 = ml_dtypes.bfloat16

B, S, D, A = 4, 4096, 128, 64
NQ = S // 2          # q rows per core
QB = 512             # q block (psum bank free size)
KC = 128             # k chunk (matmul contraction tile)
NKC = S // KC        # 32 chunks
NQB = NQ // QB       # 4 q blocks
GROUP = 2            # k chunks per exp group ([128,1024] psum tile)
NGRP = NKC // GROUP  # 16 groups per block
EXP = mybir.ActivationFunctionType.Exp

# tuning knobs
DVE_EXP_GROUPS = (1, 5, 9, 11, 13)  # groups per qblock: exp on DVE (Schraudolph)
T2_LAG = 1        # groups between a t2 tile's last input and the t2 add
DMM_LAG = 3       # groups between a t2 tile's last input and its D-matmul
FINISH_DELAY = 4  # groups into the next block before finishing a block
WARM_MMS = 8      # PE warm-up matmuls during the input-DMA wait (HAM)

# Schraudolph constants: u16 = round(s * SCH_A + SCH_B) viewed as bf16
# approximates exp(s/8).  t = s*log2(e)/8; bits = 128*t + (127*128 - C).
SCH_A = 128 * np.log2(np.e) / 8          # 23.083120654223414
SCH_B = 16256.0 - 7.5                    # C=7.5 splits round/trunc modes

_CACHED_NC = None


def _log(msg):
    import time as _t
    print(f"[kernel {_t.strftime('%H:%M:%S')}] {msg}", file=sys.stderr, flush=True)


def build_nc():
    _log("build_nc: tracing graph")
    nc = bacc.Bacc(
        "TRN2", target_bir_lowering=False, debug=False,
        enable_asserts=False, num_devices=8,
    )
    xT = nc.dram_tensor("xT", [D, S], BF16, kind="ExternalInput").ap()
    xc = nc.dram_tensor("xc", [128, S], BF16, kind="ExternalInput").ap()
    wqk = nc.dram_tensor("wqk", [D, 256], BF16, kind="ExternalInput").ap()
    wvT = nc.dram_tensor("wvT", [D, D], BF16, kind="ExternalInput").ap()
    # outT layout [v, q] f32; host transposes during gather
    out = nc.dram_tensor("out", [D, NQ], F32, kind="ExternalOutput").ap()

    with tile.TileContext(nc) as tc, ExitStack() as ctx:
        persist = ctx.enter_context(tc.tile_pool(name="persist", bufs=1))
        # PSUM: st 2x(2 banks) + px 2x(1 bank) + ms 2x(pj ring + dps ring)
        ps_st = ctx.enter_context(tc.tile_pool(name="ps_st", bufs=2, space="PSUM"))
        ps_px = ctx.enter_context(tc.tile_pool(name="ps_px", bufs=2, space="PSUM"))
        ps_ms = ctx.enter_context(tc.tile_pool(name="ps_ms", bufs=1, space="PSUM"))
        ppool = ctx.enter_context(tc.tile_pool(name="ppool", bufs=12))
        tpool = ctx.enter_context(tc.tile_pool(name="tpool", bufs=16))
        mpool = ctx.enter_context(tc.tile_pool(name="mpool", bufs=4))

        # ---- persistent SBUF ----
        wqk_s = persist.tile([D, 256], BF16, tag="wqk_s")
        wv_s = persist.tile([D, D], BF16, tag="wv_s")
        ones_s = persist.tile([128, 128], BF16, tag="ones_s")
        xT_s = persist.tile([D, S], BF16, tag="xT_s")
        xc_s = persist.tile([128, S], BF16, tag="xc_s")
        KT_s = persist.tile([128, S], BF16, tag="KT_s")   # duplicated halves
        QT_s = persist.tile([128, NQ], BF16, tag="QT_s")  # duplicated halves
        wq_s = wqk_s[:, 0:128]
        wk_s = wqk_s[:, 128:256]

        # ones for the D-matmuls needs no DMA
        nc.gpsimd.memset(ones_s[:], 1.0)
        # scratch rhs for the PE warm-up matmuls
        wscr = mpool.tile([128, QB], BF16, tag="wscr")
        nc.gpsimd.memset(wscr[:], 1.0)

        # single sync ring, consumption order (multi-ring DMA lets the
        # bulk transfers steal HBM bandwidth from the critical first ones)
        nc.sync.dma_start(wqk_s[:], wqk[:])
        nc.sync.dma_start(xT_s[:, 0:512], xT[:, 0:512])
        nc.sync.dma_start(xT_s[:, 512:1024], xT[:, 512:1024])
        nc.sync.dma_start(xc_s[:, 0:1024], xc[:, 0:1024])
        nc.sync.dma_start(xT_s[:, 1024:2048], xT[:, 1024:2048])
        nc.sync.dma_start(xc_s[:, 1024:2048], xc[:, 1024:2048])
        nc.sync.dma_start(xT_s[:, 2048:4096], xT[:, 2048:4096])
        nc.sync.dma_start(xc_s[:, 2048:4096], xc[:, 2048:4096])
        nc.sync.dma_start(wv_s[:], wvT[:])

        # prewarm the exp table (ScalarE) off the critical path
        warm = persist.tile([1, 1], F32, tag="warm")
        nc.gpsimd.memset(warm[:], 1.0)
        warm2 = persist.tile([1, 1], F32, tag="warm2")
        nc.scalar.activation(warm2[:], warm[:], EXP)

        # PE warm-up: dummy matmuls on memset data keep the PE busy during
        # the input-DMA wait so HAM un-throttles (1.2->2.4 GHz) before the
        # real stream starts; accumulation groups avoid per-MM WAR waits
        # so the stream is dense enough to trip the HAM activity window
        nacc = WARM_MMS // 2
        for wi in range(2):
            wt = ps_px.tile([128, QB], F32, tag="px", name=f"warm{wi}")
            for k in range(nacc):
                nc.tensor.matmul(wt[:], ones_s[:], wscr[:],
                                 start=(k == 0), stop=(k == nacc - 1))

        # ---- projections (just-in-time emission below for later blocks) ----
        def proj_mm(dst, w, src_slice, cp=None, pool=None):
            if pool is None:
                pt = ps_ms.tile([128, QB], F32, tag="pj", bufs=1)
            else:
                pt = pool.tile([128, QB], F32, tag="st")
            nc.tensor.matmul(pt[:], w, src_slice, start=True, stop=True)
            (cp or nc.vector.tensor_copy)(dst, pt[:])

        # prologue projections rotate through the 2-slot st pool so the
        # matmul->cast->matmul chain pipelines instead of serializing on
        # the single pj slot
        proj_mm(QT_s[:, 0:QB], wq_s, xT_s[:, 0:QB], pool=ps_st)
        proj_mm(KT_s[:, 0:QB], wk_s, xT_s[:, 0:QB], pool=ps_st)
        proj_mm(KT_s[:, QB:2 * QB], wk_s, xT_s[:, QB:2 * QB], pool=ps_st)
        kt_done = 2
        qt_done = 1

        # ramp-fill: the first ST groups are gated on cross-engine proj
        # copies (~0.7us each); these filler matmuls keep the PE dense
        # through that window so the HAM activity monitor un-throttles
        # before the steady stream begins (cold MMs run 1.5x slower)
        for wi in range(2, 4):
            wt = ps_px.tile([128, QB], F32, tag="px", name=f"warm{wi}")
            for k in range(2):
                nc.tensor.matmul(wt[:], ones_s[:], wscr[:],
                                 start=(k == 0), stop=(k == 1))

        # ---- attention: flat software pipeline over (qblock, group) ----
        ALL = [(qb, g) for qb in range(NQB) for g in range(NGRP)]

        def emit_st(qb, g):
            q0 = qb * QB
            st = ps_st.tile([128, GROUP * QB], F32, tag="st")
            for i in range(GROUP):
                kc = g * GROUP + i
                h = kc % 2  # row-tile half: concurrent 64-contraction pairs
                lhsT = KT_s[h * 64:(h + 1) * 64, kc * KC:(kc + 1) * KC]
                rhs = QT_s[h * 64:(h + 1) * 64, q0:q0 + QB]
                nc.tensor.matmul(st[:, i * QB:(i + 1) * QB], lhsT, rhs,
                                 start=True, stop=True)
            return st

        st_tiles = {}
        st_tiles[ALL[0]] = emit_st(*ALL[0])
        st_tiles[ALL[1]] = emit_st(*ALL[1])

        px_tiles = {}    # per-qblock PX^T [d, q] psum accumulators
        dps_tiles = {}   # per-qblock D psum accumulators (partition-broadcast)
        t1_tiles = {}    # (qb, g) -> bf16 chunk-pair sums (DVE)
        pending = {}     # emission idx -> list of closures (lagged work)

        t2_tiles = {}

        def emit_t2(qb, j):
            """t2 pair-sum on GpSimd (emitted early so it's long done
            before the lagged D-matmul hits the PE FIFO)."""
            t2 = tpool.tile([128, QB], BF16, bufs=8, tag="t2",
                            name=f"t2_{qb}_{j}")
            nc.gpsimd.tensor_add(t2[:], t1_tiles.pop((qb, 2 * j))[:],
                                 t1_tiles.pop((qb, 2 * j + 1))[:])
            t2_tiles[(qb, j)] = t2

        def emit_dmm(qb, j):
            """Accumulating ones-matmul folding t2 partitions into D."""
            if qb not in dps_tiles:
                dps_tiles[qb] = ps_ms.tile([128, QB], F32, tag="dps",
                                           name=f"dps{qb}", bufs=1)
            dps = dps_tiles[qb]
            nc.tensor.matmul(dps[:], ones_s[:], t2_tiles.pop((qb, j))[:],
                             start=(j == 0), stop=(j == NGRP // 2 - 1))

        def finish_block(qb, halves=1):
            q0 = qb * QB
            dps = dps_tiles.pop(qb)
            px = px_tiles.pop(qb)
            w = QB // halves
            for hf in range(halves):
                c0 = hf * w
                dinvb = mpool.tile([128, w], F32, tag="dinvb")
                nc.vector.reciprocal_approx_fast(dinvb[:], dps[:, c0:c0 + w])
                pxc = mpool.tile([128, w], BF16, tag="pxc")
                nc.scalar.copy(pxc[:], px[:, c0:c0 + w])
                pxn = mpool.tile([128, w], BF16, tag="pxn")
                nc.vector.tensor_mul(pxn[:], pxc[:], dinvb[:])
                po = ps_ms.tile([128, w], F32, tag="pj", name=f"po{qb}_{hf}",
                                bufs=1)
                nc.tensor.matmul(po[:], wv_s[:], pxn[:], start=True, stop=True)
                ot = mpool.tile([128, w], F32, tag="ot")
                nc.scalar.copy(ot[:], po[:])
                nc.sync.dma_start(out[:, q0 + c0:q0 + c0 + w], ot[:])

        for idx, (qb, g) in enumerate(ALL):
            st = st_tiles.pop((qb, g))
            p = ppool.tile([128, GROUP * QB], BF16, tag="p")
            if g in DVE_EXP_GROUPS:
                # Schraudolph exp on DVE: p_bits = s*A + B, u16-converted
                nc.vector.tensor_scalar(
                    p[:].bitcast(U16), st[:], SCH_A, SCH_B,
                    mybir.AluOpType.mult, mybir.AluOpType.add)
            else:
                # two sequential ScalarE chunk-instructions instead of one
                # slab: same engine/FIFO order, but the first PX matmul
                # gates on a 570ns chunk instead of the 1114ns slab
                nc.scalar.activation(p[:, 0:QB], st[:, 0:QB], EXP,
                                     scale=0.125)
                nc.scalar.activation(p[:, QB:2 * QB], st[:, QB:2 * QB],
                                     EXP, scale=0.125)

            if idx + 2 < len(ALL):
                st_tiles[ALL[idx + 2]] = emit_st(*ALL[idx + 2])

            if qb not in px_tiles:
                px_tiles[qb] = ps_px.tile([128, QB], F32, tag="px",
                                          name=f"px{qb}")
            px = px_tiles[qb]
            for i in range(GROUP):
                kc = g * GROUP + i
                nc.tensor.matmul(px[:], xc_s[:, kc * KC:(kc + 1) * KC],
                                 p[:, i * QB:(i + 1) * QB],
                                 start=(kc == 0), stop=(kc == NKC - 1))

            # level-1 chunk-pair sum on DVE; level-2 + D-matmul lag behind
            t1 = tpool.tile([128, QB], BF16, tag="t1")
            nc.vector.tensor_add(t1[:], p[:, 0:QB], p[:, QB:2 * QB])
            t1_tiles[(qb, g)] = t1
            if g % 2 == 1:
                j = g // 2
                pending.setdefault(idx + T2_LAG, []).append(
                    lambda qb=qb, j=j: emit_t2(qb, j))
                pending.setdefault(idx + DMM_LAG, []).append(
                    lambda qb=qb, j=j: emit_dmm(qb, j))

            # just-in-time projections: KT block j feeds ST groups 2j..2j+1
            # (emitted 2 ahead), QT block j feeds q-block j
            need_kt = min(8, (idx + 3) // 2 + 1)
            while kt_done < need_kt:
                proj_mm(KT_s[:, kt_done * QB:(kt_done + 1) * QB], wk_s,
                        xT_s[:, kt_done * QB:(kt_done + 1) * QB],
                        cp=nc.scalar.copy)
                kt_done += 1
            need_qt = min(NQB, (idx + 3) // NGRP + 1)
            while qt_done < need_qt:
                proj_mm(QT_s[:, qt_done * QB:(qt_done + 1) * QB], wq_s,
                        xT_s[:, qt_done * QB:(qt_done + 1) * QB],
                        cp=nc.scalar.copy)
                qt_done += 1

            # lagged t2-adds (GpSimd) + D-matmuls (PE) land at the END of
            # the slot so the cross-engine chain never heads-of-line the
            # PE FIFO
            for fn in pending.pop(idx, ()):
                fn()

            if g == FINISH_DELAY - 1 and qb > 0:
                finish_block(qb - 1)

        for idx in sorted(k for k in pending if k >= len(ALL)):
            for fn in pending.pop(idx):
                fn()
        finish_block(NQB - 1, halves=2)

    _log("build_nc: bacc compile")
    nc.compile()
    _log("build_nc: done")
    return nc


def _host_prep(x, Wq, Wk, Wv):
    x = np.asarray(x, dtype=np.float32)
    Wq = np.asarray(Wq, dtype=np.float32)
    Wk = np.asarray(Wk, dtype=np.float32)
    Wv = np.asarray(Wv, dtype=np.float32)
    wqk = np.concatenate(
        [Wq.T, Wq.T, Wk.T, Wk.T], axis=1).astype(NPBF16)  # [128, 256]
    wvT = np.ascontiguousarray(Wv.T).astype(NPBF16)
    in_maps = []
    for c in range(8):
        b, h = c // 2, c % 2
        # k-rotation: own q rows first, so one SPMD program fits all cores
        xr = np.roll(x[b], -h * NQ, axis=0)
        in_maps.append({
            "xT": np.ascontiguousarray(xr.T).astype(NPBF16),
            "xc": np.ascontiguousarray(
                xr.reshape(NKC, KC, D).transpose(1, 0, 2).reshape(KC, S)
            ).astype(NPBF16),
            "wqk": wqk, "wvT": wvT,
        })
    return in_maps


def run(x, Wq, Wk, Wv, trace=False, **kw):
    global _CACHED_NC
    if _CACHED_NC is None:
        _CACHED_NC = build_nc()
    in_maps = _host_prep(x, Wq, Wk, Wv)
    _log("run_bass_kernel_spmd (includes NEFF compile on first call)")
    res = run_bass_kernel_spmd(
        _CACHED_NC, in_maps, core_ids=list(range(8)), trace=trace, **kw)
    _log("run_bass_kernel_spmd returned")
    full = np.zeros((B, S, D), np.float32)
    for c in range(8):
        b, h = c // 2, c % 2
        full[b, h * NQ:(h + 1) * NQ] = np.asarray(
            res.results[c]["out"]).astype(np.float32).T
    return full, res


def kernel(x, Wq, Wk, Wv):
    full, _ = run(x, Wq, Wk, Wv, trace=False)
    return full


# revision 36
# speedup vs baseline: 1.1041x; 1.1041x over previous
"""Self-contained Trainium2 attention-block kernel (8 NeuronCores, SPMD).

Problem: x[4,4096,128], Wq/Wk[64,128], Wv[128,128] ->
  softmax((x Wq^T)(x Wk^T)^T / 8) (x Wv^T)   -> [4,4096,128] f32

Sharding: data-parallel over batch (4) x query-halves (2) = 8 cores.
Each core: q rows 2048, full K (4096) recomputed locally. No collectives.

v2 design (v1 measured 97.3us here / 99.2us on the grading harness;
this version measured 97.4us here with a colder-device-friendly start):
  - P*V reassociated as (P*X)*Wv^T: the per-chunk PV matmul uses raw x
    chunks as stationary weights, killing the V projection.
  - k-rotation: host rotates the k rows by h*2048 per core so the core's
    own q rows are always xT[:, 0:2048] -> no separate xqT tensor, one
    SPMD program for all cores (k-order is softmax/PX-invariant).
  - prologue: 9 consolidated sync-ring DMAs in consumption order (a
    second HWDGE ring was tried and let bulk transfers steal HBM
    bandwidth from the critical first slices -> slower).
  - PE warm-up: 8 dummy accumulating matmuls during the input-DMA wait
    trip the HAM activity window so the real stream starts at 2.4 GHz.
  - exp split across engines: 4 of 16 groups per q-block run on the DVE
    as a one-instruction Schraudolph bit-trick (tensor_scalar u16 =
    s*A + B, bitcast to bf16 ~= exp(s/8), ~1.2% weight rms err); the
    other 12 use ScalarE's table exp.
  - softmax denominator: DVE chunk-pair adds (t1), GpSimd pair sums
    (t2, emitted at lag 1), accumulating ones-matmuls on PE folding the
    128 k-partitions into a broadcast D row, emitted at lag 3 at the
    END of the slot body -- the cross-engine t1->t2->D chain otherwise
    head-of-line blocks the strict-FIFO PE queue (~10us of stalls).
  - finish: recip (DVE) -> px cast to bf16 (ScalarE) -> pxn mul (DVE)
    -> Wv matmul -> cast -> DMA; the last block is split into column
    halves so the tail chain pipelines across engines.
Measured: v2.3 (4 DVE slabs) 97408/97440/98346ns; v2.8 (+4
ramp-fill matmuls between the prologue projections and the first ST
groups) 96490ns; v3.0 (this config: 5 pair-second DVE slabs
(1,5,9,11,13), balancing Sc 59.1us vs DVE 60.9us busy) 93952ns, fro
rel err 4.52e-3.  HAM reaches K=8/8 at ~10.2us, right as the real
stream starts.  Near-misses: 6 ramp-fill matmuls 97304; T2_LAG=0 +
DMM_LAG=4 + FINISH_DELAY=5 regress to 110355 (the D-chain lags are a
razor-edge optimum at 1/3/4).
Schedule sensitivities found: whole-slab exp assignment beats per-slab
column-splitting (the split's DVE-queue serialization delays the st
psum WAR release and the t1 chain); DVE exp groups must be pair-second
(odd) positions or the t2 chain couples in; JIT projections must NOT
rotate through the st psum ring (off-by-one ring WAR stalls ~7us).
"""

import sys

sys.path.insert(0, "/opt/trn_rl_repo")

from contextlib import ExitStack

import ml_dtypes
import numpy as np

import concourse.bass as bass  # noqa: F401
import concourse.bacc as bacc
import concourse.tile as tile
from concourse import mybir
from concourse.bass_utils import run_bass_kernel_spmd

BF16 = mybir.dt.bfloat16
F32 = mybir.dt.float32
U16 = mybir.dt.uint16
NPBF16 = ml_dtypes.bfloat16

B, S, D, A = 4, 4096, 128, 64
NQ = S // 2          # q rows per core
QB = 512             # q block (psum bank free size)
KC = 128             # k chunk (matmul contraction tile)
NKC = S // KC        # 32 chunks
NQB = NQ // QB       # 4 q blocks
GROUP = 2            # k chunks per exp group ([128,1024] psum tile)
NGRP = NKC // GROUP  # 16 groups per block
EXP = mybir.ActivationFunctionType.Exp

# tuning knobs
DVE_EXP_GROUPS = (1, 5, 9, 11, 13)  # groups per qblock: exp on DVE (Schraudolph)
T2_LAG = 1        # groups between a t2 tile's last input and the t2 add
DMM_LAG = 3       # groups between a t2 tile's last input and its D-matmul
FINISH_DELAY = 4  # groups into the next block before finishing a block
WARM_MMS = 8      # PE warm-up matmuls during the input-DMA wait (HAM)

# Schraudolph constants: u16 = round(s * SCH_A + SCH_B) viewed as bf16
# approximates exp(s/8).  t = s*log2(e)/8; bits = 128*t + (127*128 - C).
SCH_A = 128 * np.log2(np.e) / 8          # 23.083120654223414
SCH_B = 16256.0 - 7.5                    # C=7.5 splits round/trunc modes

_CACHED_NC = None


def _log(msg):
    import time as _t
    print(f"[kernel {_t.strftime('%H:%M:%S')}] {msg}", file=sys.stderr, flush=True)


def build_nc():
    _log("build_nc: tracing graph")
    nc = bacc.Bacc(
        "TRN2", target_bir_lowering=False, debug=False,
        enable_asserts=False, num_devices=8,
    )
    xT = nc.dram_tensor("xT", [D, S], BF16, kind="ExternalInput").ap()
    xc = nc.dram_tensor("xc", [128, S], BF16, kind="ExternalInput").ap()
    wqk = nc.dram_tensor("wqk", [D, 256], BF16, kind="ExternalInput").ap()
    wvT = nc.dram_tensor("wvT", [D, D], BF16, kind="ExternalInput").ap()
    # outT layout [v, q] f32; host transposes during gather
    out = nc.dram_tensor("out", [D, NQ], F32, kind="ExternalOutput").ap()

    with tile.TileContext(nc) as tc, ExitStack() as ctx:
        persist = ctx.enter_context(tc.tile_pool(name="persist", bufs=1))
        # PSUM: st 2x(2 banks) + px 2x(1 bank) + ms 2x(pj ring + dps ring)
        ps_st = ctx.enter_context(tc.tile_pool(name="ps_st", bufs=2, space="PSUM"))
        ps_px = ctx.enter_context(tc.tile_pool(name="ps_px", bufs=2, space="PSUM"))
        ps_ms = ctx.enter_context(tc.tile_pool(name="ps_ms", bufs=1, space="PSUM"))
        ppool = ctx.enter_context(tc.tile_pool(name="ppool", bufs=12))
        tpool = ctx.enter_context(tc.tile_pool(name="tpool", bufs=16))
        mpool = ctx.enter_context(tc.tile_pool(name="mpool", bufs=4))

        # ---- persistent SBUF ----
        wqk_s = persist.tile([D, 256], BF16, tag="wqk_s")
        wv_s = persist.tile([D, D], BF16, tag="wv_s")
        ones_s = persist.tile([128, 128], BF16, tag="ones_s")
        xT_s = persist.tile([D, S], BF16, tag="xT_s")
        xc_s = persist.tile([128, S], BF16, tag="xc_s")
        KT_s = persist.tile([128, S], BF16, tag="KT_s")   # duplicated halves
        QT_s = persist.tile([128, NQ], BF16, tag="QT_s")  # duplicated halves
        wq_s = wqk_s[:, 0:128]
        wk_s = wqk_s[:, 128:256]

        # ones for the D-matmuls needs no DMA
        nc.gpsimd.memset(ones_s[:], 1.0)
        # scratch rhs for the PE warm-up matmuls
        wscr = mpool.tile([128, QB], BF16, tag="wscr")
        nc.gpsimd.memset(wscr[:], 1.0)

        # single sync ring, consumption order (multi-ring DMA lets the
        # bulk transfers steal HBM bandwidth from the critical first ones)
        nc.sync.dma_start(wqk_s[:], wqk[:])
        nc.sync.dma_start(xT_s[:, 0:512], xT[:, 0:512])
        nc.sync.dma_start(xT_s[:, 512:1024], xT[:, 512:1024])
        nc.sync.dma_start(xc_s[:, 0:1024], xc[:, 0:1024])
        nc.sync.dma_start(xT_s[:, 1024:2048], xT[:, 1024:2048])
        nc.sync.dma_start(xc_s[:, 1024:2048], xc[:, 1024:2048])
        nc.sync.dma_start(xT_s[:, 2048:4096], xT[:, 2048:4096])
        nc.sync.dma_start(xc_s[:, 2048:4096], xc[:, 2048:4096])
        nc.sync.dma_start(wv_s[:], wvT[:])

        # prewarm the exp table (ScalarE) off the critical path
        warm = persist.tile([1, 1], F32, tag="warm")
        nc.gpsimd.memset(warm[:], 1.0)
        warm2 = persist.tile([1, 1], F32, tag="warm2")
        nc.scalar.activation(warm2[:], warm[:], EXP)

        # PE warm-up: dummy matmuls on memset data keep the PE busy during
        # the input-DMA wait so HAM un-throttles (1.2->2.4 GHz) before the
        # real stream starts; accumulation groups avoid per-MM WAR waits
        # so the stream is dense enough to trip the HAM activity window
        nacc = WARM_MMS // 2
        for wi in range(2):
            wt = ps_px.tile([128, QB], F32, tag="px", name=f"warm{wi}")
            for k in range(nacc):
                nc.tensor.matmul(wt[:], ones_s[:], wscr[:],
                                 start=(k == 0), stop=(k == nacc - 1))

        # ---- projections (just-in-time emission below for later blocks) ----
        def proj_mm(dst, w, src_slice, cp=None, pool=None):
            if pool is None:
                pt = ps_ms.tile([128, QB], F32, tag="pj", bufs=1)
            else:
                pt = pool.tile([128, QB], F32, tag="st")
            nc.tensor.matmul(pt[:], w, src_slice, start=True, stop=True)
            (cp or nc.vector.tensor_copy)(dst, pt[:])

        # prologue projections rotate through the 2-slot st pool so the
        # matmul->cast->matmul chain pipelines instead of serializing on
        # the single pj slot
        proj_mm(QT_s[:, 0:QB], wq_s, xT_s[:, 0:QB], pool=ps_st)
        proj_mm(KT_s[:, 0:QB], wk_s, xT_s[:, 0:QB], pool=ps_st)
        proj_mm(KT_s[:, QB:2 * QB], wk_s, xT_s[:, QB:2 * QB], pool=ps_st)
        kt_done = 2
        qt_done = 1

        # ramp-fill: the first ST groups are gated on cross-engine proj
        # copies (~0.7us each); these filler matmuls keep the PE dense
        # through that window so the HAM activity monitor un-throttles
        # before the steady stream begins (cold MMs run 1.5x slower)
        for wi in range(2, 4):
            wt = ps_px.tile([128, QB], F32, tag="px", name=f"warm{wi}")
            for k in range(2):
                nc.tensor.matmul(wt[:], ones_s[:], wscr[:],
                                 start=(k == 0), stop=(k == 1))

        # ---- attention: flat software pipeline over (qblock, group) ----
        ALL = [(qb, g) for qb in range(NQB) for g in range(NGRP)]

        def emit_st(qb, g):
            q0 = qb * QB
            st = ps_st.tile([128, GROUP * QB], F32, tag="st")
            for i in range(GROUP):
                kc = g * GROUP + i
                h = kc % 2  # row-tile half: concurrent 64-contraction pairs
                lhsT = KT_s[h * 64:(h + 1) * 64, kc * KC:(kc + 1) * KC]
                rhs = QT_s[h * 64:(h + 1) * 64, q0:q0 + QB]
                nc.tensor.matmul(st[:, i * QB:(i + 1) * QB], lhsT, rhs,
                                 start=True, stop=True)
            return st

        st_tiles = {}
        st_tiles[ALL[0]] = emit_st(*ALL[0])
        st_tiles[ALL[1]] = emit_st(*ALL[1])

        px_tiles = {}    # per-qblock PX^T [d, q] psum accumulators
        dps_tiles = {}   # per-qblock D psum accumulators (partition-broadcast)
        t1_tiles = {}    # (qb, g) -> bf16 chunk-pair sums (DVE)
        pending = {}     # emission idx -> list of closures (lagged work)

        t2_tiles = {}

        def emit_t2(qb, j):
            """t2 pair-sum on GpSimd (emitted early so it's long done
            before the lagged D-matmul hits the PE FIFO)."""
            t2 = tpool.tile([128, QB], BF16, bufs=8, tag="t2",
                            name=f"t2_{qb}_{j}")
            nc.gpsimd.tensor_add(t2[:], t1_tiles.pop((qb, 2 * j))[:],
                                 t1_tiles.pop((qb, 2 * j + 1))[:])
            t2_tiles[(qb, j)] = t2

        def emit_dmm(qb, j):
            """Accumulating ones-matmul folding t2 partitions into D."""
            if qb not in dps_tiles:
                dps_tiles[qb] = ps_ms.tile([128, QB], F32, tag="dps",
                                           name=f"dps{qb}", bufs=1)
            dps = dps_tiles[qb]
            nc.tensor.matmul(dps[:], ones_s[:], t2_tiles.pop((qb, j))[:],
                             start=(j == 0), stop=(j == NGRP // 2 - 1))

        def emit_dmm_direct(qb):
            """Last pair of the last block: fold t1s into D with two
            direct ones-matmuls, skipping the 1.17us GpSimd t2 hop on
            the serial tail chain."""
            j = NGRP // 2 - 1
            a = t1_tiles.pop((qb, 2 * j))
            b = t1_tiles.pop((qb, 2 * j + 1))
            dps = dps_tiles[qb]
            nc.tensor.matmul(dps[:], ones_s[:], a[:], start=False, stop=False)
            nc.tensor.matmul(dps[:], ones_s[:], b[:], start=False, stop=True)

        def finish_block(qb, halves=1):
            q0 = qb * QB
            dps = dps_tiles.pop(qb)
            px = px_tiles.pop(qb)
            w = QB // halves
            for hf in range(halves):
                c0 = hf * w
                dinvb = mpool.tile([128, w], F32, tag="dinvb")
                nc.vector.reciprocal_approx_fast(dinvb[:], dps[:, c0:c0 + w])
                pxc = mpool.tile([128, w], BF16, tag="pxc")
                nc.scalar.copy(pxc[:], px[:, c0:c0 + w])
                pxn = mpool.tile([128, w], BF16, tag="pxn")
                nc.vector.tensor_mul(pxn[:], pxc[:], dinvb[:])
                po = ps_ms.tile([128, w], F32, tag="pj", name=f"po{qb}_{hf}",
                                bufs=1)
                nc.tensor.matmul(po[:], wv_s[:], pxn[:], start=True, stop=True)
                ot = mpool.tile([128, w], F32, tag="ot")
                nc.scalar.copy(ot[:], po[:])
                nc.sync.dma_start(out[:, q0 + c0:q0 + c0 + w], ot[:])

        for idx, (qb, g) in enumerate(ALL):
            st = st_tiles.pop((qb, g))
            p = ppool.tile([128, GROUP * QB], BF16, tag="p")
            if g in DVE_EXP_GROUPS:
                # Schraudolph exp on DVE: p_bits = s*A + B, u16-converted
                nc.vector.tensor_scalar(
                    p[:].bitcast(U16), st[:], SCH_A, SCH_B,
                    mybir.AluOpType.mult, mybir.AluOpType.add)
            else:
                nc.scalar.activation(p[:], st[:], EXP, scale=0.125)

            if idx + 2 < len(ALL):
                st_tiles[ALL[idx + 2]] = emit_st(*ALL[idx + 2])

            if qb not in px_tiles:
                px_tiles[qb] = ps_px.tile([128, QB], F32, tag="px",
                                          name=f"px{qb}")
            px = px_tiles[qb]
            for i in range(GROUP):
                kc = g * GROUP + i
                nc.tensor.matmul(px[:], xc_s[:, kc * KC:(kc + 1) * KC],
                                 p[:, i * QB:(i + 1) * QB],
                                 start=(kc == 0), stop=(kc == NKC - 1))

            # level-1 chunk-pair sum on DVE; level-2 + D-matmul lag behind
            t1 = tpool.tile([128, QB], BF16, tag="t1")
            nc.vector.tensor_add(t1[:], p[:, 0:QB], p[:, QB:2 * QB])
            t1_tiles[(qb, g)] = t1
            if g % 2 == 1:
                j = g // 2
                if qb == NQB - 1 and j == NGRP // 2 - 1:
                    pending.setdefault(idx + T2_LAG, []).append(
                        lambda qb=qb: emit_dmm_direct(qb))
                else:
                    pending.setdefault(idx + T2_LAG, []).append(
                        lambda qb=qb, j=j: emit_t2(qb, j))
                    pending.setdefault(idx + DMM_LAG, []).append(
                        lambda qb=qb, j=j: emit_dmm(qb, j))

            # just-in-time projections: KT block j feeds ST groups 2j..2j+1
            # (emitted 2 ahead), QT block j feeds q-block j
            need_kt = min(8, (idx + 3) // 2 + 1)
            while kt_done < need_kt:
                proj_mm(KT_s[:, kt_done * QB:(kt_done + 1) * QB], wk_s,
                        xT_s[:, kt_done * QB:(kt_done + 1) * QB],
                        cp=nc.scalar.copy)
                kt_done += 1
            need_qt = min(NQB, (idx + 3) // NGRP + 1)
            while qt_done < need_qt:
                proj_mm(QT_s[:, qt_done * QB:(qt_done + 1) * QB], wq_s,
                        xT_s[:, qt_done * QB:(qt_done + 1) * QB],
                        cp=nc.scalar.copy)
                qt_done += 1

            # lagged t2-adds (GpSimd) + D-matmuls (PE) land at the END of
            # the slot so the cross-engine chain never heads-of-line the
            # PE FIFO
            for fn in pending.pop(idx, ()):
                fn()

            if g == FINISH_DELAY - 1 and qb > 0:
                finish_block(qb - 1)

        for idx in sorted(k for k in pending if k >= len(ALL)):
            for fn in pending.pop(idx):
                fn()
        finish_block(NQB - 1, halves=2)

    _log("build_nc: bacc compile")
    nc.compile()
    _log("build_nc: done")
    return nc


def _host_prep(x, Wq, Wk, Wv):
    x = np.asarray(x, dtype=np.float32)
    Wq = np.asarray(Wq, dtype=np.float32)
    Wk = np.asarray(Wk, dtype=np.float32)
    Wv = np.asarray(Wv, dtype=np.float32)
    wqk = np.concatenate(
        [Wq.T, Wq.T, Wk.T, Wk.T], axis=1).astype(NPBF16)  # [128, 256]
    wvT = np.ascontiguousarray(Wv.T).astype(NPBF16)
    in_maps = []
    for c in range(8):
        b, h = c // 2, c % 2
        # k-rotation: own q rows first, so one SPMD program fits all cores
        xr = np.roll(x[b], -h * NQ, axis=0)
        in_maps.append({
            "xT": np.ascontiguousarray(xr.T).astype(NPBF16),
            "xc": np.ascontiguousarray(
                xr.reshape(NKC, KC, D).transpose(1, 0, 2).reshape(KC, S)
            ).astype(NPBF16),
            "wqk": wqk, "wvT": wvT,
        })
    return in_maps


def run(x, Wq, Wk, Wv, trace=False, **kw):
    global _CACHED_NC
    if _CACHED_NC is None:
        _CACHED_NC = build_nc()
    in_maps = _host_prep(x, Wq, Wk, Wv)
    _log("run_bass_kernel_spmd (includes NEFF compile on first call)")
    res = run_bass_kernel_spmd(
        _CACHED_NC, in_maps, core_ids=list(range(8)), trace=trace, **kw)
    _log("run_bass_kernel_spmd returned")
    full = np.zeros((B, S, D), np.float32)
    for c in range(8):
        b, h = c // 2, c % 2
        full[b, h * NQ:(h + 1) * NQ] = np.asarray(
            res.results[c]["out"]).astype(np.float32).T
    return full, res


def kernel(x, Wq, Wk, Wv):
    full, _ = run(x, Wq, Wk, Wv, trace=False)
    return full
